# revision 1
# baseline (speedup 1.0000x reference)
"""APRConv (gnn_message_passing) Trainium2 kernel.

Strategy (8 NeuronCores, data-parallel over particles):
  - Particle dim N is sharded 8 ways; the intensities table [N, 8]
    (fp16, channel-last) is replicated to every core so all neighbor
    gathers are core-local (random neighbor indices make halos
    pointless).
  - Host sorts each core's particles by stencil id (stable), so every
    512-particle tile uses a single stencil. The per-tile [72, 8]
    weight slice is DMA'd from a host-prepared sequence; no one-hot
    mask stream, no mask multiply, no stencil-selection matmul. The
    host inverse-permutes output columns at the end (pure layout work).
  - Per 512-particle tile: 36 indirect DMA gathers fetch the 512*9
    neighbor rows (one 16-byte fp16 row per partition per instruction
    -- the HW contract for indirect DMA is one dynamic index per
    partition; wider offset APs fetch consecutive rows / crash).
  - Gathered [128, 288] particle-major fp16 tiles are transposed on
    the PE (identity matmul) into G_t [72, 512] = (tap, chan) x
    particle; one fp16 matmul against the tile's [72, 8] weight slice
    produces all 8 output channels; bias is added on the Vector engine
    during the PSUM->SBUF copy.
fp16 keeps the gather payload at 16 B/row (half the f32 DMA bytes) and
its 2^-11 mantissa keeps the worst-case relative error ~1e-3, well
inside the 2e-2 gate.
Host-side work is limited to sharding, layout transforms (transpose,
index reshuffle, stencil sort, weight-slice sequencing) and the final
inverse permutation + concat.
"""
import numpy as np

import concourse.bass as bass
import concourse.tile as tile
from concourse import mybir
from concourse.bass import IndirectOffsetOnAxis
from concourse.bass_utils import run_bass_kernel_spmd

B, CIN, COUT = 1, 8, 8
N = 2_000_000
K2 = 9
S = 4
NCORES = 8
NP_CORE = N // NCORES            # 250_000 particles per core
TILE = 512                       # particles per tile
IPT = (TILE // 128) * K2         # 36 indices per partition per tile
JC = K2 * CIN                    # 72 (tap, chan) rows
# per-stencil-group padding to tile boundary can add up to S-1 tiles
NTILES = (NP_CORE + TILE - 1) // TILE + (S - 1)   # 492
NPAD = NTILES * TILE

_CACHE = {}


def _split_drain_waits(nc, max_waits=1):
    """This walrus build rejects instructions with >1 sem wait; move excess
    waits onto same-engine nops inserted just before the instruction.

    Also drop the kernel-tail EVENT_SEMAPHORE_RANGE_CLEAR InstISA (walrus
    codegen here rejects InstISA); it is redundant with the is_reset_sema
    Drain emitted immediately before it, which resets the same sem range.
    """
    blocks = nc.main_func.blocks
    for bb in blocks:
        for ins in list(bb.instructions):
            if type(ins).__name__ == "InstISA" and ins.isa_opcode == 176:
                si = ins.sync_info
                assert si is None or (not si.on_wait and not si.on_update)
                bb.instructions.remove(ins)

    def fresh_nop(engine):
        nop = nc.engines[engine].nop(nofuse=True).ins
        for b in blocks:
            if b.instructions and b.instructions[-1] is nop:
                b.instructions.pop()
                return nop
        raise AssertionError("appended nop not found at any block tail")

    for bb in blocks:
        if not any(
            ins.sync_info is not None and len(ins.sync_info.on_wait) > max_waits
            for ins in bb.instructions
        ):
            continue
        new_list = []
        for ins in list(bb.instructions):
            si = ins.sync_info
            if si is not None and len(si.on_wait) > max_waits:
                waits = list(si.on_wait)
                si.on_wait = waits[:max_waits]
                for w in waits[max_waits:]:
                    nop = fresh_nop(ins.engine)
                    nop.sync_info = mybir.SyncInfo(on_wait=[w], on_update=[])
                    new_list.append(nop)
            new_list.append(ins)
        bb.instructions[:] = new_list


def _build_nc(CHUNK=99, IDXB=8, GB=6):
    nc = bass.Bass()
    f32 = mybir.dt.float32
    f16 = mybir.dt.float16
    table = nc.declare_dram_parameter("table", [N, CIN], f16, isOutput=False)
    idxd = nc.declare_dram_parameter("idxd", [128, NTILES * IPT], mybir.dt.int32, isOutput=False)
    wseq = nc.declare_dram_parameter("wseq", [JC, NTILES * COUT], f16, isOutput=False)
    biasd = nc.declare_dram_parameter("biasd", [COUT, 1], f32, isOutput=False)
    identd = nc.declare_dram_parameter("identd", [128, 128], f16, isOutput=False)
    outd = nc.declare_dram_parameter("outd", [COUT, NPAD], f32, isOutput=True)

    # Keep semaphore ids distinct across the sequential TileContexts below:
    # freeing + clearing between contexts emits EVENT_SEMAPHORE_RANGE_CLEAR
    # (InstISA), which this walrus build cannot codegen; with fresh ids per
    # context no clearing is needed at all. CHUNK=99 keeps it to 5 contexts
    # (6+ exhausts the sem pool; ~128+ tiles hits a Tile-scheduler cliff).
    nc.clear_and_free_semaphores = lambda sems: None

    for c0 in range(0, NTILES, CHUNK):
        with tile.TileContext(nc) as tc, (
            tc.tile_pool(name="consts", bufs=1)
        ) as constp, (
            tc.tile_pool(name="idxp", bufs=IDXB)
        ) as idxp, (
            tc.tile_pool(name="gp", bufs=GB)
        ) as gp, (
            tc.tile_pool(name="gtp", bufs=3)
        ) as gtp, (
            tc.tile_pool(name="wp", bufs=3)
        ) as wp, (
            tc.tile_pool(name="outp", bufs=3)
        ) as outp, (
            tc.tile_pool(name="psA", bufs=4, space="PSUM")
        ) as psA, (
            tc.tile_pool(name="psC", bufs=2, space="PSUM")
        ) as psC:
            ident = constp.tile([128, 128], f16)
            nc.sync.dma_start(ident[:], identd[:])
            bt = constp.tile([COUT, 1], f32)
            nc.sync.dma_start(bt[:], biasd[:])

            for t in range(c0, min(c0 + CHUNK, NTILES)):
                it = idxp.tile([128, IPT], mybir.dt.int32)
                nc.sync.dma_start(it[:], idxd[:, t * IPT:(t + 1) * IPT])
                g = gp.tile([128, IPT * CIN], f16)
                for k in range(IPT):
                    nc.gpsimd.indirect_dma_start(
                        out=g[:, k * CIN:(k + 1) * CIN],
                        out_offset=None,
                        in_=table[:],
                        in_offset=IndirectOffsetOnAxis(ap=it[:, k:k + 1], axis=0),
                    )
                wt = wp.tile([JC, COUT], f16)
                nc.sync.dma_start(wt[:], wseq[:, t * COUT:(t + 1) * COUT])
                gt = gtp.tile([JC, TILE], f16)
                for q in range(TILE // 128):
                    ps = psA.tile([JC, 128], f16)
                    nc.tensor.transpose(
                        out=ps[:], in_=g[:, q * JC:(q + 1) * JC], identity=ident[:]
                    )
                    nc.scalar.copy(gt[:, q * 128:(q + 1) * 128], ps[:])
                ops = psC.tile([COUT, TILE], f32)
                nc.tensor.matmul(ops[:], lhsT=wt[:], rhs=gt[:], start=True, stop=True)
                ot = outp.tile([COUT, TILE], f32)
                nc.vector.tensor_scalar_add(ot[:], ops[:], bt[:])
                nc.sync.dma_start(outd[:, t * TILE:(t + 1) * TILE], ot[:])

    _split_drain_waits(nc)
    return nc


def _prepare(intensities, weight, bias, nbr_idx, stencil_ids):
    """Host-side sharding + layout prep. Returns (in_maps, perms); perms[c]
    maps kernel output column -> particle offset within core c (-1 pad)."""
    table = np.ascontiguousarray(
        intensities.reshape(CIN, N).T.astype(np.float16)
    )  # [N, 8]
    nbr = np.asarray(nbr_idx, dtype=np.int32)         # [N, 9]
    sid = np.asarray(stencil_ids, dtype=np.int32)     # [N]

    wr = np.asarray(weight, dtype=np.float32).reshape(COUT, CIN, S, K2)
    w2 = [
        np.ascontiguousarray(
            wr[:, :, s, :].transpose(2, 1, 0).reshape(JC, COUT)
        ).astype(np.float16)
        for s in range(S)
    ]
    biasc = np.asarray(bias, dtype=np.float32).reshape(COUT, 1)
    ident = np.eye(128, dtype=np.float16)

    in_maps = []
    perms = []
    for c in range(NCORES):
        lo = c * NP_CORE
        sid_c = sid[lo:lo + NP_CORE]
        order = np.argsort(sid_c, kind="stable")
        counts = np.bincount(sid_c, minlength=S)
        slot_perm = np.full(NPAD, -1, np.int64)
        tile_sten = np.zeros(NTILES, np.int32)
        slot = 0
        pos = 0
        for s in range(S):
            n_s = int(counts[s])
            ntile_s = (n_s + TILE - 1) // TILE
            slot_perm[slot:slot + n_s] = order[pos:pos + n_s]
            for tt in range(ntile_s):
                tile_sten[slot // TILE + tt] = s
            slot += ntile_s * TILE
            pos += n_s

        nb = np.zeros((NPAD, K2), np.int32)
        valid = slot_perm >= 0
        nb[valid] = nbr[lo + slot_perm[valid]]
        idxd = np.ascontiguousarray(
            nb.reshape(NTILES, TILE // 128, 128, K2)
            .transpose(2, 0, 1, 3)
            .reshape(128, NTILES * IPT)
        )
        wseq = np.ascontiguousarray(
            np.concatenate([w2[tile_sten[t]] for t in range(NTILES)], axis=1)
        )
        in_maps.append(
            {
                "table": table,
                "idxd": idxd,
                "wseq": wseq,
                "biasd": biasc,
                "identd": ident,
            }
        )
        perms.append(slot_perm)
    return in_maps, perms


def _unshard(results, perms):
    out = np.empty((B, COUT, N), np.float32)
    for c in range(NCORES):
        sp = perms[c]
        valid = sp >= 0
        out[0, :, c * NP_CORE + sp[valid]] = results[c]["outd"][:, valid].T
    return out


def _get_nc():
    if "nc" not in _CACHE:
        _CACHE["nc"] = _build_nc()
    return _CACHE["nc"]


def kernel(intensities, weight, bias, nbr_idx, stencil_ids):
    nc = _get_nc()
    in_maps, perms = _prepare(intensities, weight, bias, nbr_idx, stencil_ids)
    res = run_bass_kernel_spmd(nc, in_maps, list(range(NCORES)))
    return _unshard(res.results, perms)



# revision 4
# speedup vs baseline: 1.3888x; 1.3888x over previous
"""APRConv (gnn_message_passing) Trainium2 kernel.

Strategy (8 NeuronCores, data-parallel over particles):
  - Particle dim N is sharded 8 ways; the intensities table [N, 8]
    (fp16, channel-last) is replicated to every core so all neighbor
    gathers are core-local (random neighbor indices make halos
    pointless).
  - Host sorts each core's particles by stencil id (stable), so every
    512-particle tile uses a single stencil. The per-tile [72, 8]
    weight slice is DMA'd from a host-prepared sequence; no one-hot
    mask stream, no mask multiply, no stencil-selection matmul. The
    host inverse-permutes output columns at the end (pure layout work).
  - Per 512-particle tile: 36 indirect DMA gathers fetch the 512*9
    neighbor rows (one 16-byte fp16 row per partition per instruction
    -- the HW contract for indirect DMA is one dynamic index per
    partition; wider offset APs fetch consecutive rows / crash).
  - Gathered [128, 288] particle-major fp16 tiles are transposed on
    the PE (identity matmul) into G_t [72, 512] = (tap, chan) x
    particle; one fp16 matmul against the tile's [72, 8] weight slice
    produces all 8 output channels; bias is added on the Vector engine
    during the PSUM->SBUF copy.
fp16 keeps the gather payload at 16 B/row (half the f32 DMA bytes) and
its 2^-11 mantissa keeps the worst-case relative error ~1e-3, well
inside the 2e-2 gate.
Host-side work is limited to sharding, layout transforms (transpose,
index reshuffle, stencil sort, weight-slice sequencing) and the final
inverse permutation + concat.
"""
import numpy as np

import concourse.bass as bass
import concourse.tile as tile
from concourse import mybir
from concourse.bass import IndirectOffsetOnAxis
from concourse.bass_utils import run_bass_kernel_spmd

B, CIN, COUT = 1, 8, 8
N = 2_000_000
K2 = 9
S = 4
NCORES = 8
NP_CORE = N // NCORES            # 250_000 particles per core
TILE = 512                       # particles per tile
IPT = (TILE // 128) * K2         # 36 indices per partition per tile
JC = K2 * CIN                    # 72 (tap, chan) rows
# per-stencil-group padding to tile boundary can add up to S-1 tiles
NTILES = (NP_CORE + TILE - 1) // TILE + (S - 1)   # 492
NPAD = NTILES * TILE

_CACHE = {}


def _split_drain_waits(nc, max_waits=1):
    """This walrus build rejects instructions with >1 sem wait; move excess
    waits onto same-engine nops inserted just before the instruction.

    Also drop the kernel-tail EVENT_SEMAPHORE_RANGE_CLEAR InstISA (walrus
    codegen here rejects InstISA); it is redundant with the is_reset_sema
    Drain emitted immediately before it, which resets the same sem range.
    """
    blocks = nc.main_func.blocks
    for bb in blocks:
        for ins in list(bb.instructions):
            if type(ins).__name__ == "InstISA" and ins.isa_opcode == 176:
                si = ins.sync_info
                assert si is None or (not si.on_wait and not si.on_update)
                bb.instructions.remove(ins)

    def fresh_nop(engine):
        nop = nc.engines[engine].nop(nofuse=True).ins
        for b in blocks:
            if b.instructions and b.instructions[-1] is nop:
                b.instructions.pop()
                return nop
        raise AssertionError("appended nop not found at any block tail")

    for bb in blocks:
        if not any(
            ins.sync_info is not None and len(ins.sync_info.on_wait) > max_waits
            for ins in bb.instructions
        ):
            continue
        new_list = []
        for ins in list(bb.instructions):
            si = ins.sync_info
            if si is not None and len(si.on_wait) > max_waits:
                waits = list(si.on_wait)
                si.on_wait = waits[:max_waits]
                for w in waits[max_waits:]:
                    nop = fresh_nop(ins.engine)
                    nop.sync_info = mybir.SyncInfo(on_wait=[w], on_update=[])
                    new_list.append(nop)
            new_list.append(ins)
        bb.instructions[:] = new_list


def _build_nc(CHUNK=99, IDXB=8, GB=6):
    nc = bass.Bass()
    f32 = mybir.dt.float32
    f16 = mybir.dt.float16
    table = nc.declare_dram_parameter("table", [N, CIN], f16, isOutput=False)
    idxd = nc.declare_dram_parameter("idxd", [128, NTILES * IPT], mybir.dt.int32, isOutput=False)
    wseq = nc.declare_dram_parameter("wseq", [JC, NTILES * COUT], f16, isOutput=False)
    biasd = nc.declare_dram_parameter("biasd", [COUT, 1], f32, isOutput=False)
    identd = nc.declare_dram_parameter("identd", [128, 128], f16, isOutput=False)
    outd = nc.declare_dram_parameter("outd", [COUT, NPAD], f32, isOutput=True)

    # Keep semaphore ids distinct across the sequential TileContexts below:
    # freeing + clearing between contexts emits EVENT_SEMAPHORE_RANGE_CLEAR
    # (InstISA), which this walrus build cannot codegen; with fresh ids per
    # context no clearing is needed at all. CHUNK=99 keeps it to 5 contexts
    # (6+ exhausts the sem pool; ~128+ tiles hits a Tile-scheduler cliff).
    nc.clear_and_free_semaphores = lambda sems: None

    for c0 in range(0, NTILES, CHUNK):
        with tile.TileContext(nc) as tc, (
            tc.tile_pool(name="consts", bufs=1)
        ) as constp, (
            tc.tile_pool(name="idxp", bufs=IDXB)
        ) as idxp, (
            tc.tile_pool(name="gp", bufs=GB)
        ) as gp, (
            tc.tile_pool(name="gtp", bufs=3)
        ) as gtp, (
            tc.tile_pool(name="wp", bufs=3)
        ) as wp, (
            tc.tile_pool(name="outp", bufs=3)
        ) as outp, (
            tc.tile_pool(name="psA", bufs=4, space="PSUM")
        ) as psA, (
            tc.tile_pool(name="psC", bufs=2, space="PSUM")
        ) as psC:
            ident = constp.tile([128, 128], f16)
            nc.sync.dma_start(ident[:], identd[:])
            bt = constp.tile([COUT, 1], f32)
            nc.sync.dma_start(bt[:], biasd[:])

            for t in range(c0, min(c0 + CHUNK, NTILES)):
                it = idxp.tile([128, IPT], mybir.dt.int32)
                nc.sync.dma_start(it[:], idxd[:, t * IPT:(t + 1) * IPT])
                g = gp.tile([128, IPT * CIN], f16)
                for k in range(IPT):
                    nc.gpsimd.indirect_dma_start(
                        out=g[:, k * CIN:(k + 1) * CIN],
                        out_offset=None,
                        in_=table[:],
                        in_offset=IndirectOffsetOnAxis(ap=it[:, k:k + 1], axis=0),
                    )
                wt = wp.tile([JC, COUT], f16)
                nc.sync.dma_start(wt[:], wseq[:, t * COUT:(t + 1) * COUT])
                gt = gtp.tile([JC, TILE], f16)
                for q in range(TILE // 128):
                    ps = psA.tile([JC, 128], f16)
                    nc.tensor.transpose(
                        out=ps[:], in_=g[:, q * JC:(q + 1) * JC], identity=ident[:]
                    )
                    nc.scalar.copy(gt[:, q * 128:(q + 1) * 128], ps[:])
                ops = psC.tile([COUT, TILE], f32)
                nc.tensor.matmul(ops[:], lhsT=wt[:], rhs=gt[:], start=True, stop=True)
                ot = outp.tile([COUT, TILE], f32)
                nc.vector.tensor_scalar_add(ot[:], ops[:], bt[:])
                nc.sync.dma_start(outd[:, t * TILE:(t + 1) * TILE], ot[:])

    _split_drain_waits(nc)
    return nc


def _prepare(intensities, weight, bias, nbr_idx, stencil_ids):
    """Host-side sharding + layout prep. Returns (in_maps, perms); perms[c]
    maps kernel output column -> particle offset within core c (-1 pad)."""
    table = np.ascontiguousarray(
        intensities.reshape(CIN, N).T.astype(np.float16)
    )  # [N, 8]
    nbr = np.asarray(nbr_idx, dtype=np.int32)         # [N, 9]
    sid = np.asarray(stencil_ids, dtype=np.int32)     # [N]

    wr = np.asarray(weight, dtype=np.float32).reshape(COUT, CIN, S, K2)
    w2 = [
        np.ascontiguousarray(
            wr[:, :, s, :].transpose(2, 1, 0).reshape(JC, COUT)
        ).astype(np.float16)
        for s in range(S)
    ]
    biasc = np.asarray(bias, dtype=np.float32).reshape(COUT, 1)
    ident = np.eye(128, dtype=np.float16)

    in_maps = []
    perms = []
    for c in range(NCORES):
        lo = c * NP_CORE
        sid_c = sid[lo:lo + NP_CORE]
        order = np.argsort(sid_c, kind="stable")
        counts = np.bincount(sid_c, minlength=S)
        slot_perm = np.full(NPAD, -1, np.int64)
        tile_sten = np.zeros(NTILES, np.int32)
        slot = 0
        pos = 0
        for s in range(S):
            n_s = int(counts[s])
            ntile_s = (n_s + TILE - 1) // TILE
            slot_perm[slot:slot + n_s] = order[pos:pos + n_s]
            for tt in range(ntile_s):
                tile_sten[slot // TILE + tt] = s
            slot += ntile_s * TILE
            pos += n_s

        nb = np.zeros((NPAD, K2), np.int32)
        valid = slot_perm >= 0
        nb[valid] = nbr[lo + slot_perm[valid]]
        idxd = np.ascontiguousarray(
            nb.reshape(NTILES, TILE // 128, 128, K2)
            .transpose(2, 0, 1, 3)
            .reshape(128, NTILES * IPT)
        )
        wseq = np.ascontiguousarray(
            np.concatenate([w2[tile_sten[t]] for t in range(NTILES)], axis=1)
        )
        in_maps.append(
            {
                "table": table,
                "idxd": idxd,
                "wseq": wseq,
                "biasd": biasc,
                "identd": ident,
            }
        )
        perms.append(slot_perm)
    return in_maps, perms


def _unshard(results, perms):
    out = np.empty((B, COUT, N), np.float32)
    for c in range(NCORES):
        sp = perms[c]
        valid = sp >= 0
        out[0, :, c * NP_CORE + sp[valid]] = results[c]["outd"][:, valid].T
    return out


def _get_nc():
    if "nc" not in _CACHE:
        _CACHE["nc"] = _build_nc()
    return _CACHE["nc"]


def kernel(intensities, weight, bias, nbr_idx, stencil_ids):
    nc = _get_nc()
    in_maps, perms = _prepare(intensities, weight, bias, nbr_idx, stencil_ids)
    res = run_bass_kernel_spmd(nc, in_maps, list(range(NCORES)))
    return _unshard(res.results, perms)



# revision 5
# speedup vs baseline: 1.4465x; 1.0416x over previous
"""APRConv (gnn_message_passing) Trainium2 kernel.

Strategy (8 NeuronCores, data-parallel over particles):
  - Particle dim N is sharded 8 ways; the intensities table [N, 8]
    (fp16, channel-last) is replicated to every core so all neighbor
    gathers are core-local (random neighbor indices make halos
    pointless).
  - Host sorts each core's particles by stencil id (stable), so every
    512-particle tile uses a single stencil. The per-tile [72, 8]
    weight slice is DMA'd from a host-prepared sequence; no one-hot
    mask stream, no mask multiply, no stencil-selection matmul. The
    host inverse-permutes output columns at the end (pure layout work).
  - Per 512-particle tile: 36 indirect DMA gathers fetch the 512*9
    neighbor rows (one 16-byte fp16 row per partition per instruction
    -- the HW contract for indirect DMA is one dynamic index per
    partition; wider offset APs fetch consecutive rows / crash).
  - Gathered [128, 288] particle-major fp16 tiles are transposed on
    the PE (identity matmul) into G_t [72, 512] = (tap, chan) x
    particle; one fp16 matmul against the tile's [72, 8] weight slice
    produces all 8 output channels; bias is added on the Vector engine
    during the PSUM->SBUF copy.
fp16 keeps the gather payload at 16 B/row (half the f32 DMA bytes) and
its 2^-11 mantissa keeps the worst-case relative error ~1e-3, well
inside the 2e-2 gate.
Host-side work is limited to sharding, layout transforms (transpose,
index reshuffle, stencil sort, weight-slice sequencing) and the final
inverse permutation + concat.
"""
import numpy as np

import concourse.bass as bass
import concourse.tile as tile
from concourse import mybir
from concourse.bass import IndirectOffsetOnAxis
from concourse.bass_utils import run_bass_kernel_spmd

B, CIN, COUT = 1, 8, 8
N = 2_000_000
K2 = 9
S = 4
NCORES = 8
NP_CORE = N // NCORES            # 250_000 particles per core
TILE = 512                       # particles per tile
IPT = (TILE // 128) * K2         # 36 indices per partition per tile
JC = K2 * CIN                    # 72 (tap, chan) rows
# per-stencil-group padding to tile boundary can add up to S-1 tiles
NTILES = (NP_CORE + TILE - 1) // TILE + (S - 1)   # 492
NPAD = NTILES * TILE

_CACHE = {}


def _split_drain_waits(nc, max_waits=1):
    """This walrus build rejects instructions with >1 sem wait; move excess
    waits onto same-engine nops inserted just before the instruction.

    Also drop the kernel-tail EVENT_SEMAPHORE_RANGE_CLEAR InstISA (walrus
    codegen here rejects InstISA); it is redundant with the is_reset_sema
    Drain emitted immediately before it, which resets the same sem range.
    """
    blocks = nc.main_func.blocks
    for bb in blocks:
        for ins in list(bb.instructions):
            if type(ins).__name__ == "InstISA" and ins.isa_opcode == 176:
                si = ins.sync_info
                assert si is None or (not si.on_wait and not si.on_update)
                bb.instructions.remove(ins)

    def fresh_nop(engine):
        nop = nc.engines[engine].nop(nofuse=True).ins
        for b in blocks:
            if b.instructions and b.instructions[-1] is nop:
                b.instructions.pop()
                return nop
        raise AssertionError("appended nop not found at any block tail")

    for bb in blocks:
        if not any(
            ins.sync_info is not None and len(ins.sync_info.on_wait) > max_waits
            for ins in bb.instructions
        ):
            continue
        new_list = []
        for ins in list(bb.instructions):
            si = ins.sync_info
            if si is not None and len(si.on_wait) > max_waits:
                waits = list(si.on_wait)
                si.on_wait = waits[:max_waits]
                for w in waits[max_waits:]:
                    nop = fresh_nop(ins.engine)
                    nop.sync_info = mybir.SyncInfo(on_wait=[w], on_update=[])
                    new_list.append(nop)
            new_list.append(ins)
        bb.instructions[:] = new_list


def _build_nc(CHUNK=123, IDXB=10, GB=8):
    nc = bass.Bass()
    f32 = mybir.dt.float32
    f16 = mybir.dt.float16
    table = nc.declare_dram_parameter("table", [N, CIN], f16, isOutput=False)
    idxd = nc.declare_dram_parameter("idxd", [128, NTILES * IPT], mybir.dt.int32, isOutput=False)
    wseq = nc.declare_dram_parameter("wseq", [JC, NTILES * COUT], f16, isOutput=False)
    biasd = nc.declare_dram_parameter("biasd", [COUT, 1], f32, isOutput=False)
    identd = nc.declare_dram_parameter("identd", [128, 128], f16, isOutput=False)
    outd = nc.declare_dram_parameter("outd", [COUT, NPAD], f32, isOutput=True)

    # Keep semaphore ids distinct across the sequential TileContexts below:
    # freeing + clearing between contexts emits EVENT_SEMAPHORE_RANGE_CLEAR
    # (InstISA), which this walrus build cannot codegen; with fresh ids per
    # context no clearing is needed at all. CHUNK=99 keeps it to 5 contexts
    # (6+ exhausts the sem pool; ~128+ tiles hits a Tile-scheduler cliff).
    nc.clear_and_free_semaphores = lambda sems: None

    for c0 in range(0, NTILES, CHUNK):
        with tile.TileContext(nc) as tc, (
            tc.tile_pool(name="consts", bufs=1)
        ) as constp, (
            tc.tile_pool(name="idxp", bufs=IDXB)
        ) as idxp, (
            tc.tile_pool(name="gp", bufs=GB)
        ) as gp, (
            tc.tile_pool(name="gtp", bufs=3)
        ) as gtp, (
            tc.tile_pool(name="wp", bufs=3)
        ) as wp, (
            tc.tile_pool(name="outp", bufs=3)
        ) as outp, (
            tc.tile_pool(name="psA", bufs=4, space="PSUM")
        ) as psA, (
            tc.tile_pool(name="psC", bufs=2, space="PSUM")
        ) as psC:
            ident = constp.tile([128, 128], f16)
            nc.sync.dma_start(ident[:], identd[:])
            bt = constp.tile([COUT, 1], f32)
            nc.sync.dma_start(bt[:], biasd[:])

            for t in range(c0, min(c0 + CHUNK, NTILES)):
                it = idxp.tile([128, IPT], mybir.dt.int32)
                nc.sync.dma_start(it[:], idxd[:, t * IPT:(t + 1) * IPT])
                g = gp.tile([128, IPT * CIN], f16)
                for k in range(IPT):
                    nc.gpsimd.indirect_dma_start(
                        out=g[:, k * CIN:(k + 1) * CIN],
                        out_offset=None,
                        in_=table[:],
                        in_offset=IndirectOffsetOnAxis(ap=it[:, k:k + 1], axis=0),
                    )
                wt = wp.tile([JC, COUT], f16)
                nc.sync.dma_start(wt[:], wseq[:, t * COUT:(t + 1) * COUT])
                gt = gtp.tile([JC, TILE], f16)
                for q in range(TILE // 128):
                    ps = psA.tile([JC, 128], f16)
                    nc.tensor.transpose(
                        out=ps[:], in_=g[:, q * JC:(q + 1) * JC], identity=ident[:]
                    )
                    nc.scalar.copy(gt[:, q * 128:(q + 1) * 128], ps[:])
                ops = psC.tile([COUT, TILE], f32)
                nc.tensor.matmul(ops[:], lhsT=wt[:], rhs=gt[:], start=True, stop=True)
                ot = outp.tile([COUT, TILE], f32)
                nc.vector.tensor_scalar_add(ot[:], ops[:], bt[:])
                nc.sync.dma_start(outd[:, t * TILE:(t + 1) * TILE], ot[:])

    _split_drain_waits(nc)
    return nc


def _prepare(intensities, weight, bias, nbr_idx, stencil_ids):
    """Host-side sharding + layout prep. Returns (in_maps, perms); perms[c]
    maps kernel output column -> particle offset within core c (-1 pad)."""
    table = np.ascontiguousarray(
        intensities.reshape(CIN, N).T.astype(np.float16)
    )  # [N, 8]
    nbr = np.asarray(nbr_idx, dtype=np.int32)         # [N, 9]
    sid = np.asarray(stencil_ids, dtype=np.int32)     # [N]

    wr = np.asarray(weight, dtype=np.float32).reshape(COUT, CIN, S, K2)
    w2 = [
        np.ascontiguousarray(
            wr[:, :, s, :].transpose(2, 1, 0).reshape(JC, COUT)
        ).astype(np.float16)
        for s in range(S)
    ]
    biasc = np.asarray(bias, dtype=np.float32).reshape(COUT, 1)
    ident = np.eye(128, dtype=np.float16)

    in_maps = []
    perms = []
    for c in range(NCORES):
        lo = c * NP_CORE
        sid_c = sid[lo:lo + NP_CORE]
        order = np.argsort(sid_c, kind="stable")
        counts = np.bincount(sid_c, minlength=S)
        slot_perm = np.full(NPAD, -1, np.int64)
        tile_sten = np.zeros(NTILES, np.int32)
        slot = 0
        pos = 0
        for s in range(S):
            n_s = int(counts[s])
            ntile_s = (n_s + TILE - 1) // TILE
            slot_perm[slot:slot + n_s] = order[pos:pos + n_s]
            for tt in range(ntile_s):
                tile_sten[slot // TILE + tt] = s
            slot += ntile_s * TILE
            pos += n_s

        nb = np.zeros((NPAD, K2), np.int32)
        valid = slot_perm >= 0
        nb[valid] = nbr[lo + slot_perm[valid]]
        idxd = np.ascontiguousarray(
            nb.reshape(NTILES, TILE // 128, 128, K2)
            .transpose(2, 0, 1, 3)
            .reshape(128, NTILES * IPT)
        )
        wseq = np.ascontiguousarray(
            np.concatenate([w2[tile_sten[t]] for t in range(NTILES)], axis=1)
        )
        in_maps.append(
            {
                "table": table,
                "idxd": idxd,
                "wseq": wseq,
                "biasd": biasc,
                "identd": ident,
            }
        )
        perms.append(slot_perm)
    return in_maps, perms


def _unshard(results, perms):
    out = np.empty((B, COUT, N), np.float32)
    for c in range(NCORES):
        sp = perms[c]
        valid = sp >= 0
        out[0, :, c * NP_CORE + sp[valid]] = results[c]["outd"][:, valid].T
    return out


def _get_nc():
    if "nc" not in _CACHE:
        _CACHE["nc"] = _build_nc()
    return _CACHE["nc"]


def kernel(intensities, weight, bias, nbr_idx, stencil_ids):
    nc = _get_nc()
    in_maps, perms = _prepare(intensities, weight, bias, nbr_idx, stencil_ids)
    res = run_bass_kernel_spmd(nc, in_maps, list(range(NCORES)))
    return _unshard(res.results, perms)



# revision 6
# speedup vs baseline: 1.4649x; 1.0127x over previous
"""APRConv (gnn_message_passing) Trainium2 kernel.

Strategy (8 NeuronCores, data-parallel over particles):
  - Particle dim N is sharded 8 ways; the intensities table [N, 8]
    (fp16, channel-last) is replicated to every core so all neighbor
    gathers are core-local (random neighbor indices make halos
    pointless).
  - Host sorts each core's particles by stencil id (stable), so every
    512-particle tile uses a single stencil. The per-tile [72, 8]
    weight slice is DMA'd from a host-prepared sequence; no one-hot
    mask stream, no mask multiply, no stencil-selection matmul. The
    host inverse-permutes output columns at the end (pure layout work).
  - Per 512-particle tile: 36 indirect DMA gathers fetch the 512*9
    neighbor rows (one 16-byte fp16 row per partition per instruction
    -- the HW contract for indirect DMA is one dynamic index per
    partition; wider offset APs fetch consecutive rows / crash).
  - Gathered [128, 288] particle-major fp16 tiles are transposed on
    the PE (identity matmul) into G_t [72, 512] = (tap, chan) x
    particle; one fp16 matmul against the tile's [72, 8] weight slice
    produces all 8 output channels; bias is added on the Vector engine
    during the PSUM->SBUF copy.
fp16 keeps the gather payload at 16 B/row (half the f32 DMA bytes) and
its 2^-11 mantissa keeps the worst-case relative error ~1e-3, well
inside the 2e-2 gate.
Host-side work is limited to sharding, layout transforms (transpose,
index reshuffle, stencil sort, weight-slice sequencing) and the final
inverse permutation + concat.
"""
import numpy as np

import concourse.bass as bass
import concourse.tile as tile
from concourse import mybir
from concourse.bass import IndirectOffsetOnAxis
from concourse.bass_utils import run_bass_kernel_spmd

B, CIN, COUT = 1, 8, 8
N = 2_000_000
K2 = 9
S = 4
NCORES = 8
NP_CORE = N // NCORES            # 250_000 particles per core
TILE = 512                       # particles per tile
IPT = (TILE // 128) * K2         # 36 indices per partition per tile
JC = K2 * CIN                    # 72 (tap, chan) rows
# per-stencil-group padding to tile boundary can add up to S-1 tiles
NTILES = (NP_CORE + TILE - 1) // TILE + (S - 1)   # 492
NPAD = NTILES * TILE

_CACHE = {}


def _split_drain_waits(nc, max_waits=1):
    """This walrus build rejects instructions with >1 sem wait; move excess
    waits onto same-engine nops inserted just before the instruction.

    Also drop the kernel-tail EVENT_SEMAPHORE_RANGE_CLEAR InstISA (walrus
    codegen here rejects InstISA); it is redundant with the is_reset_sema
    Drain emitted immediately before it, which resets the same sem range.
    """
    blocks = nc.main_func.blocks
    for bb in blocks:
        for ins in list(bb.instructions):
            if type(ins).__name__ == "InstISA" and ins.isa_opcode == 176:
                si = ins.sync_info
                assert si is None or (not si.on_wait and not si.on_update)
                bb.instructions.remove(ins)

    def fresh_nop(engine):
        nop = nc.engines[engine].nop(nofuse=True).ins
        for b in blocks:
            if b.instructions and b.instructions[-1] is nop:
                b.instructions.pop()
                return nop
        raise AssertionError("appended nop not found at any block tail")

    for bb in blocks:
        if not any(
            ins.sync_info is not None and len(ins.sync_info.on_wait) > max_waits
            for ins in bb.instructions
        ):
            continue
        new_list = []
        for ins in list(bb.instructions):
            si = ins.sync_info
            if si is not None and len(si.on_wait) > max_waits:
                waits = list(si.on_wait)
                si.on_wait = waits[:max_waits]
                for w in waits[max_waits:]:
                    nop = fresh_nop(ins.engine)
                    nop.sync_info = mybir.SyncInfo(on_wait=[w], on_update=[])
                    new_list.append(nop)
            new_list.append(ins)
        bb.instructions[:] = new_list


def _build_nc(CHUNK=99, IDXB=8, GB=6):
    nc = bass.Bass()
    f32 = mybir.dt.float32
    f16 = mybir.dt.float16
    table = nc.declare_dram_parameter("table", [N, CIN], f16, isOutput=False)
    idxd = nc.declare_dram_parameter("idxd", [128, NTILES * IPT], mybir.dt.int32, isOutput=False)
    wseq = nc.declare_dram_parameter("wseq", [JC, NTILES * COUT], f16, isOutput=False)
    biasd = nc.declare_dram_parameter("biasd", [COUT, 1], f32, isOutput=False)
    identd = nc.declare_dram_parameter("identd", [128, 128], f16, isOutput=False)
    outd = nc.declare_dram_parameter("outd", [COUT, NPAD], f32, isOutput=True)

    # Keep semaphore ids distinct across the sequential TileContexts below:
    # freeing + clearing between contexts emits EVENT_SEMAPHORE_RANGE_CLEAR
    # (InstISA), which this walrus build cannot codegen; with fresh ids per
    # context no clearing is needed at all. CHUNK=99 keeps it to 5 contexts
    # (6+ exhausts the sem pool; ~128+ tiles hits a Tile-scheduler cliff).
    nc.clear_and_free_semaphores = lambda sems: None

    for c0 in range(0, NTILES, CHUNK):
        with tile.TileContext(nc) as tc, (
            tc.tile_pool(name="consts", bufs=1)
        ) as constp, (
            tc.tile_pool(name="idxp", bufs=IDXB)
        ) as idxp, (
            tc.tile_pool(name="gp", bufs=GB)
        ) as gp, (
            tc.tile_pool(name="gtp", bufs=3)
        ) as gtp, (
            tc.tile_pool(name="wp", bufs=3)
        ) as wp, (
            tc.tile_pool(name="outp", bufs=3)
        ) as outp, (
            tc.tile_pool(name="psA", bufs=4, space="PSUM")
        ) as psA, (
            tc.tile_pool(name="psC", bufs=2, space="PSUM")
        ) as psC:
            ident = constp.tile([128, 128], f16)
            nc.sync.dma_start(ident[:], identd[:])
            bt = constp.tile([COUT, 1], f32)
            nc.sync.dma_start(bt[:], biasd[:])

            for t in range(c0, min(c0 + CHUNK, NTILES)):
                it = idxp.tile([128, IPT], mybir.dt.int32)
                nc.sync.dma_start(it[:], idxd[:, t * IPT:(t + 1) * IPT])
                g = gp.tile([128, IPT * CIN], f16)
                for k in range(IPT):
                    nc.gpsimd.indirect_dma_start(
                        out=g[:, k * CIN:(k + 1) * CIN],
                        out_offset=None,
                        in_=table[:],
                        in_offset=IndirectOffsetOnAxis(ap=it[:, k:k + 1], axis=0),
                    )
                wt = wp.tile([JC, COUT], f16)
                nc.sync.dma_start(wt[:], wseq[:, t * COUT:(t + 1) * COUT])
                gt = gtp.tile([JC, TILE], f16)
                for q in range(TILE // 128):
                    ps = psA.tile([JC, 128], f16)
                    nc.tensor.transpose(
                        out=ps[:], in_=g[:, q * JC:(q + 1) * JC], identity=ident[:]
                    )
                    nc.scalar.copy(gt[:, q * 128:(q + 1) * 128], ps[:])
                ops = psC.tile([COUT, TILE], f32)
                nc.tensor.matmul(ops[:], lhsT=wt[:], rhs=gt[:], start=True, stop=True)
                ot = outp.tile([COUT, TILE], f32)
                nc.vector.tensor_scalar_add(ot[:], ops[:], bt[:])
                nc.sync.dma_start(outd[:, t * TILE:(t + 1) * TILE], ot[:])

    _split_drain_waits(nc)
    return nc


def _prepare(intensities, weight, bias, nbr_idx, stencil_ids):
    """Host-side sharding + layout prep. Returns (in_maps, perms); perms[c]
    maps kernel output column -> particle offset within core c (-1 pad)."""
    table = np.ascontiguousarray(
        intensities.reshape(CIN, N).T.astype(np.float16)
    )  # [N, 8]
    nbr = np.asarray(nbr_idx, dtype=np.int32)         # [N, 9]
    sid = np.asarray(stencil_ids, dtype=np.int32)     # [N]

    wr = np.asarray(weight, dtype=np.float32).reshape(COUT, CIN, S, K2)
    w2 = [
        np.ascontiguousarray(
            wr[:, :, s, :].transpose(2, 1, 0).reshape(JC, COUT)
        ).astype(np.float16)
        for s in range(S)
    ]
    biasc = np.asarray(bias, dtype=np.float32).reshape(COUT, 1)
    ident = np.eye(128, dtype=np.float16)

    in_maps = []
    perms = []
    for c in range(NCORES):
        lo = c * NP_CORE
        sid_c = sid[lo:lo + NP_CORE]
        order = np.argsort(sid_c, kind="stable")
        counts = np.bincount(sid_c, minlength=S)
        slot_perm = np.full(NPAD, -1, np.int64)
        tile_sten = np.zeros(NTILES, np.int32)
        slot = 0
        pos = 0
        for s in range(S):
            n_s = int(counts[s])
            ntile_s = (n_s + TILE - 1) // TILE
            slot_perm[slot:slot + n_s] = order[pos:pos + n_s]
            for tt in range(ntile_s):
                tile_sten[slot // TILE + tt] = s
            slot += ntile_s * TILE
            pos += n_s

        nb = np.zeros((NPAD, K2), np.int32)
        valid = slot_perm >= 0
        nb[valid] = nbr[lo + slot_perm[valid]]
        idxd = np.ascontiguousarray(
            nb.reshape(NTILES, TILE // 128, 128, K2)
            .transpose(2, 0, 1, 3)
            .reshape(128, NTILES * IPT)
        )
        wseq = np.ascontiguousarray(
            np.concatenate([w2[tile_sten[t]] for t in range(NTILES)], axis=1)
        )
        in_maps.append(
            {
                "table": table,
                "idxd": idxd,
                "wseq": wseq,
                "biasd": biasc,
                "identd": ident,
            }
        )
        perms.append(slot_perm)
    return in_maps, perms


def _unshard(results, perms):
    out = np.empty((B, COUT, N), np.float32)
    for c in range(NCORES):
        sp = perms[c]
        valid = sp >= 0
        out[0, :, c * NP_CORE + sp[valid]] = results[c]["outd"][:, valid].T
    return out


def _get_nc():
    if "nc" not in _CACHE:
        _CACHE["nc"] = _build_nc()
    return _CACHE["nc"]


def kernel(intensities, weight, bias, nbr_idx, stencil_ids):
    nc = _get_nc()
    in_maps, perms = _prepare(intensities, weight, bias, nbr_idx, stencil_ids)
    res = run_bass_kernel_spmd(nc, in_maps, list(range(NCORES)))
    return _unshard(res.results, perms)

